# revision 9
# baseline (speedup 1.0000x reference)
"""CaptionBase greedy GRU decode on 8 Trainium2 NeuronCores.

Sharding: proposal axis P=128 split 8 ways -> 16 proposals x 4 batch = 64
rows per core.  Each core runs the full 31-step greedy decode for its rows
and writes its [31, 64, V] logits slab; the host reassembles [B, P, 31, V].

Design (PE-roofline driven):

1. The GRU input-side matmul gi = x @ W_ih.T is gone: x is always an
   embedding-table row, so the host precomputes gi = emb_table @ W_ih.T
   + b_ih as three per-gate tables ([V+B, H] fp32 each: r, z, n) and the
   kernel gathers the pre-activations by token id (indirect DMA), in the
   order the gate math consumes them.  The SOS step uses B virtual table
   rows (ids V..V+B-1) so step 0 is identical to every other step.
2. The classifier runs ONE fp16 pass (h1 @ w1), padded to 3584 columns
   so the moving-operand pitch stays aligned.  The ~3e-4-sigma logit
   error easily passes the 2e-2 output gate; only the greedy ARGMAX
   needs fp32-grade precision.  That decision is made by exactly
   rescoring the approximate top-3 candidates: gather their W_cls
   columns (fp32) and dot with h on the Vector engine in fp32 (error
   ~1e-6, the same grade as a 3-pass PE split, which matched the
   reference argmax 15872/15872).
3. The recurrent matmul gh = h @ W_hh.T keeps the 3-pass hi/lo split
   (h1(fp16)@w1(fp16) + h1(bf16)@w2(bf16) + h2(bf16)@w1(bf16)); h must
   stay ~2e-6-accurate because every future logit depends on it.
"""

import numpy as np
import ml_dtypes

import concourse.bass as bass
import concourse.bacc as bacc
import concourse.mybir as mybir
from concourse.tile import TileContext
from concourse.masks import make_identity
from concourse.bass_utils import run_bass_kernel_spmd

B, P, T, E, F, H, V = 4, 128, 32, 300, 2048, 512, 3433
NSTEP = T - 1          # 31 decode steps
NCORES = 8
PL = P // NCORES       # 16 proposals per core
R = PL * B             # 64 rows per core
KH, KF = 4, 16         # k-chunks for H/F contractions
G3 = 3 * H             # 1536
NV = (V + 511) // 512  # 7 vocab chunks
VP = NV * 512          # V padded to 3584 for aligned matmul pitch
NCAND = 3              # exactly-rescored argmax candidates

f32 = mybir.dt.float32
f16 = mybir.dt.float16
bf16 = mybir.dt.bfloat16
u32 = mybir.dt.uint32
AFT = mybir.ActivationFunctionType
ALU = mybir.AluOpType

FP16_MIN_NORMAL = 2.0 ** -14
_CACHE = {}


def _split3(w):
    """fp32 array -> (w1 fp16, w2 bf16, w1b bf16) with w ~= w1 + w2.

    fp16 subnormals are pre-flushed on the host so the PE and numpy agree
    on w1; the bf16 residual absorbs whatever was flushed.
    """
    w = np.ascontiguousarray(w, dtype=np.float32)
    w1 = w.astype(np.float16)
    w1[np.abs(w1.astype(np.float32)) < FP16_MIN_NORMAL] = 0
    w2 = (w - w1.astype(np.float32)).astype(ml_dtypes.bfloat16)
    w1b = w1.astype(ml_dtypes.bfloat16)
    return w1, w2, w1b


def _pack_chunks(a, kchunks):
    """[kchunks*128, R] -> [128, kchunks*R] with chunk c at cols c*R:(c+1)*R."""
    k128, r = a.shape
    assert k128 == kchunks * 128
    out = np.empty((128, kchunks * r), dtype=a.dtype)
    for c in range(kchunks):
        out[:, c * R:(c + 1) * R] = a[c * 128:(c + 1) * 128]
    return np.ascontiguousarray(out)


def _build_program(nonzero_bias, nstep=NSTEP):
    nc = bacc.Bacc("TRN2", target_bir_lowering=False)

    def din(name, shape, dt):
        return nc.dram_tensor(name, shape, dt, kind="ExternalInput")

    # GRU recurrent weight splits (moving operands), shared across cores.
    whh = [din(f"whh{i}", [H, G3], d) for i, d in enumerate((f16, bf16, bf16))]
    # Classifier: padded fp16 head (approx pass) + fp32 transpose (rescore).
    wcl1 = din("wcl1", [H, VP], f16)
    wclsT = din("wclsT", [V, H], f32)
    # h0 mapping weights, streamed during setup.
    wmp = [din(f"wmp{i}", [F, H], d) for i, d in enumerate((f16, bf16, bf16))]
    # Stationary setup operand, packed [128, k*R]; obj differs per core.
    obj = [din(f"obj{i}", [128, KF * R], d) for i, d in enumerate((f16, bf16, bf16))]
    # Per-gate input-side pre-activation tables (+B virtual SOS rows).
    gi_dram = {g: din(f"gi_{g}", [V + B, H], f32) for g in ("r", "z", "n")}
    idx0 = din("idx0", [R, 1], u32)
    biases = {}
    for bname, blen in (("b_rz_hh", 2 * H), ("b_hn", H), ("b_cls", V),
                        ("b_map", H)):
        if nonzero_bias.get(bname):
            biases[bname] = din(bname, [1, blen], f32)
    if nonzero_bias.get("b_cls"):
        biases["b_clsT"] = din("b_clsT", [V, 1], f32)
    out_dram = nc.dram_tensor("out", [nstep, R, V], f32, kind="ExternalOutput")

    with TileContext(nc) as tc:
        with (
            tc.tile_pool(name="const", bufs=1) as const,
            tc.tile_pool(name="wpool", bufs=1) as wpool,
            tc.tile_pool(name="work", bufs=2) as work,
            tc.tile_pool(name="psum", bufs=1, space="PSUM") as psum,
            tc.tile_pool(name="psum2", bufs=2, space="PSUM") as psum2,
        ):
            ident = const.tile([128, 128], f32)
            make_identity(nc, ident)
            # 16-bit identities: h transposes run in 16-bit PE mode (an
            # fp32_mode matmul leaves the PE streaming at half rate for the
            # next ~13 matmuls, which was costing ~3.3us/step in the cls).
            ident16 = const.tile([128, 128], f16)
            nc.vector.tensor_copy(ident16, ident)
            identb = const.tile([128, 128], bf16)
            nc.vector.tensor_copy(identb, ident)

            bias_t = {}
            for bname, ap in biases.items():
                if bname == "b_clsT":
                    continue
                blen = ap.shape[1]
                bt = const.tile([R, blen], f32, name=f"{bname}_t")
                nc.sync.dma_start(out=bt, in_=ap.to_broadcast([R, blen]))
                bias_t[bname] = bt

            # Resident weight tiles: [128, G3] / [128, VP] row-chunks.
            whh_t = [[wpool.tile([128, G3], w.dtype, name=f"whh{i}_{c}")
                      for c in range(KH)] for i, w in enumerate(whh)]
            wcl_t = [wpool.tile([128, VP], f16, name=f"wcl_{c}")
                     for c in range(KH)]
            for i in range(3):
                for c in range(KH):
                    nc.sync.dma_start(out=whh_t[i][c],
                                      in_=whh[i][c * 128:(c + 1) * 128, :])
            for c in range(KH):
                nc.sync.dma_start(out=wcl_t[c],
                                  in_=wcl1[c * 128:(c + 1) * 128, :])

            # Persistent logits tile; pad columns primed once to -inf so the
            # per-chunk MAX8 never selects them.
            logits = const.tile([R, VP], f32, name="logits")
            nc.vector.memset(logits[:, V:], -1.0e30)

            # Step-0 token ids (virtual rows V..V+B-1 of the gi tables).
            idx0_t = const.tile([R, 1], u32, name="idx0_t")
            nc.sync.dma_start(out=idx0_t, in_=idx0[:, :])

            def gather_gi(idx_ap):
                """Indirect-gather the three per-gate pre-activation rows,
                in the order the gate math consumes them."""
                gi = {}
                for g in ("r", "n", "z"):
                    gt = work.tile([R, H], f32, tag=f"gi{g}")
                    nc.gpsimd.indirect_dma_start(
                        out=gt, out_offset=None, in_=gi_dram[g][:, :],
                        in_offset=bass.IndirectOffsetOnAxis(ap=idx_ap, axis=0))
                    gi[g] = gt
                return gi

            gi_cur = gather_gi(idx0_t[:, 0:1])

            # --- h0 = relu(obj_feats @ W_map), streaming W_map chunks.
            with tc.tile_pool(name="setup", bufs=3) as setup_pool:
                obj_t = [setup_pool.tile([128, KF * R], o.dtype, bufs=1,
                                         name=f"obj_t{i}")
                         for i, o in enumerate(obj)]
                for i in range(3):
                    nc.sync.dma_start(out=obj_t[i], in_=obj[i][:, :])
                h0_ps = psum.tile([R, H], f32, tag="hnps")
                nmm = 3 * KF
                mi = 0
                # W_map streamed in 8 rounds of 2 k-chunks per version.
                for rd in range(8):
                    wm_t = [setup_pool.tile([128, 2 * H], w.dtype, tag=f"wm{i}",
                                            name=f"wm{i}_{rd}", bufs=2)
                            for i, w in enumerate(wmp)]
                    for i in range(3):
                        nc.gpsimd.dma_start(
                            out=wm_t[i][:].rearrange("p (a n) -> p a n", a=2),
                            in_=wmp[i][256 * rd:256 * (rd + 1), :].rearrange(
                                "(a p) n -> p a n", p=128))
                    for cc in range(2):
                        c = rd * 2 + cc
                        # pass pairing: (o1,w1), (o1b,w2), (o2b,w1b)
                        for ia, iw in ((0, 0), (2, 1), (1, 2)):
                            nc.tensor.matmul(h0_ps,
                                             lhsT=obj_t[ia][:, c * R:(c + 1) * R],
                                             rhs=wm_t[iw][:, cc * H:(cc + 1) * H],
                                             start=(mi == 0),
                                             stop=(mi == nmm - 1),
                                             skip_group_check=True)
                            mi += 1
                h_cur = work.tile([R, H], f32, tag="h")
                if "b_map" in bias_t:
                    nc.vector.tensor_add(h0_ps, h0_ps, bias_t["b_map"])
                nc.scalar.activation(h_cur, h0_ps, AFT.Relu)

            def transpose_split_h(h_ap, last):
                # Split h in row layout, then transpose the 16-bit parts so
                # the PE never enters fp32 mode.
                h1 = work.tile([R, H], f16, tag="h1")
                nc.vector.tensor_copy(h1, h_ap)
                hT1_ps = psum.tile([128, KH * R], f16, tag="trps")
                for c in range(KH):
                    nc.tensor.transpose(out=hT1_ps[:, c * R:(c + 1) * R],
                                        in_=h1[:, c * 128:(c + 1) * 128],
                                        identity=ident16[:R, :R])
                hT1 = work.tile([128, KH * R], f16, tag="hT1")
                nc.scalar.copy(hT1, hT1_ps)
                if last:    # bf16 splits only feed the next step's gh
                    return hT1, None, None
                h2b = work.tile([R, H], bf16, tag="h2b")
                nc.vector.tensor_sub(h2b, h_ap, h1)
                hT2_ps = psum.tile([128, KH * R], bf16, tag="tr2ps")
                for c in range(KH):
                    nc.tensor.transpose(out=hT2_ps[:, c * R:(c + 1) * R],
                                        in_=h2b[:, c * 128:(c + 1) * 128],
                                        identity=identb[:R, :R])
                hT2b = work.tile([128, KH * R], bf16, tag="hT2b")
                nc.scalar.copy(hT2b, hT2_ps)
                hT1b = work.tile([128, KH * R], bf16, tag="hT1b")
                nc.scalar.copy(hT1b, hT1_ps)
                return hT1, hT2b, hT1b

            hT1, hT2b, hT1b = transpose_split_h(h_cur, last=False)

            for t in range(nstep):
                # --- gate pre-activations: gh only (gi comes gathered) ----
                # rz_ps[:, :H] = h_r ; rz_ps[:, H:] = h_z ; hn_ps = h_n
                rz_ps = psum.tile([R, 2 * H], f32, tag="rzps")
                hn_ps = psum.tile([R, H], f32, tag="hnps")
                # pass pairing: (h1,w1) fp16, (h1b,w2b) bf16, (h2b,w1b) bf16
                gh_passes = [(hT1, whh_t[0]), (hT1b, whh_t[1]), (hT2b, whh_t[2])]
                for half in range(2):
                    sl = slice(half * H, (half + 1) * H)
                    n0 = half * H
                    mi = 0
                    for lh, rts in gh_passes:
                        for c in range(KH):
                            nc.tensor.matmul(
                                rz_ps[:, sl], lhsT=lh[:, c * R:(c + 1) * R],
                                rhs=rts[c][:, n0:n0 + H], start=(mi == 0),
                                stop=(mi == 3 * KH - 1), skip_group_check=True)
                            mi += 1
                mi = 0
                for lh, rts in gh_passes:
                    for c in range(KH):
                        nc.tensor.matmul(
                            hn_ps, lhsT=lh[:, c * R:(c + 1) * R],
                            rhs=rts[c][:, 2 * H:], start=(mi == 0),
                            stop=(mi == 3 * KH - 1), skip_group_check=True)
                        mi += 1

                # --- gates, ordered so each gathered gi part is consumed
                # as soon as it lands (r first, then n, then z) -----------
                if "b_hn" in bias_t:
                    nc.vector.tensor_add(hn_ps, hn_ps, bias_t["b_hn"])
                nc.vector.tensor_add(rz_ps[:, :H], rz_ps[:, :H], gi_cur["r"])
                if "b_rz_hh" in bias_t:
                    nc.vector.tensor_add(rz_ps, rz_ps, bias_t["b_rz_hh"])
                rz_sb = work.tile([R, 2 * H], f32, tag="rzsb", bufs=1)
                nc.scalar.activation(rz_sb[:, :H], rz_ps[:, :H], AFT.Sigmoid)
                tmp = work.tile([R, H], f32, tag="tmp")
                nc.vector.tensor_mul(tmp, rz_sb[:, :H], hn_ps)      # r * h_n
                nc.vector.tensor_add(tmp, tmp, gi_cur["n"])         # + i_n
                n_sb = work.tile([R, H], f32, tag="n")
                nc.scalar.activation(n_sb, tmp, AFT.Tanh)
                nc.vector.tensor_add(rz_ps[:, H:], rz_ps[:, H:], gi_cur["z"])
                nc.scalar.activation(rz_sb[:, H:], rz_ps[:, H:], AFT.Sigmoid)
                d_sb = work.tile([R, H], f32, tag="d")
                nc.vector.tensor_sub(d_sb, h_cur, n_sb)             # h - n
                nc.vector.tensor_mul(d_sb, rz_sb[:, H:], d_sb)      # z * (h - n)
                h_new = work.tile([R, H], f32, tag="h")
                nc.vector.tensor_add(h_new, n_sb, d_sb)             # n + z*(h-n)
                h_cur = h_new

                hT1, hT2b, hT1b = transpose_split_h(h_cur, last=(t == nstep - 1))

                # --- logits ~= h1 @ w1 (single fp16 pass, padded pitch) --
                maxes = work.tile([R, 8 * NV], f32, tag="maxes")
                for v in range(NV):
                    n0 = v * 512
                    w = min(512, V - n0)
                    cls_ps = psum2.tile([R, 512], f32, tag="clsps")
                    for c in range(KH):
                        nc.tensor.matmul(
                            cls_ps, lhsT=hT1[:, c * R:(c + 1) * R],
                            rhs=wcl_t[c][:, n0:n0 + 512], start=(c == 0),
                            stop=(c == KH - 1), skip_group_check=True)
                    if "b_cls" in bias_t:
                        nc.vector.tensor_add(logits[:, n0:n0 + w], cls_ps[:, :w],
                                             bias_t["b_cls"][:, n0:n0 + w])
                    else:
                        nc.scalar.copy(logits[:, n0:n0 + w], cls_ps[:, :w])
                    nc.vector.max(maxes[:, v * 8:(v + 1) * 8],
                                  logits[:, n0:n0 + 512])

                nc.sync.dma_start(out=out_dram[t, :, :], in_=logits[:, :V])

                if t == nstep - 1:
                    continue
                # --- approx top-8 -> exact rescore of top-NCAND ----------
                gmax = work.tile([R, 8], f32, tag="gmax")
                nc.vector.max(gmax, maxes)
                idx = work.tile([R, 8], u32, tag="idx")
                nc.vector.max_index(idx, gmax, logits[:, :V])

                # Gather candidates' fp32 classifier columns; fused
                # multiply-reduce gives each exact score as soon as its
                # gather lands.
                resc = work.tile([R, NCAND, H], f32, tag="resc")
                scr = work.tile([R, H], f32, tag="scr")
                scores = work.tile([R, NCAND], f32, tag="scores")
                for k in range(NCAND):
                    nc.gpsimd.indirect_dma_start(
                        out=resc[:, k, :], out_offset=None, in_=wclsT[:, :],
                        in_offset=bass.IndirectOffsetOnAxis(ap=idx[:, k:k + 1],
                                                            axis=0))
                for k in range(NCAND):
                    nc.vector.affine_mul_reduce(
                        out=scr, accum_out=scores[:, k:k + 1],
                        in0=resc[:, k, :], in1=h_cur, scale=1.0, bias=0.0)
                if "b_clsT" in biases:
                    bg = work.tile([R, NCAND], f32, tag="bg")
                    for k in range(NCAND):
                        nc.gpsimd.indirect_dma_start(
                            out=bg[:, k:k + 1], out_offset=None,
                            in_=biases["b_clsT"][:, :],
                            in_offset=bass.IndirectOffsetOnAxis(
                                ap=idx[:, k:k + 1], axis=0))
                    nc.vector.tensor_add(scores, scores, bg)

                # Winner: max exact score; ties keep the earlier (higher-
                # approx) candidate, matching argmax-first-occurrence.
                best_s = work.tile([R, 1], f32, tag="bs")
                best_i = work.tile([R, 1], u32, tag="bi")
                nc.vector.tensor_copy(best_s, scores[:, 0:1])
                nc.vector.tensor_copy(best_i, idx[:, 0:1])
                m = work.tile([R, 1], u32, tag="m")
                for k in range(1, NCAND):
                    nc.vector.tensor_tensor(m, scores[:, k:k + 1], best_s,
                                            op=ALU.is_gt)
                    nc.vector.copy_predicated(best_i, m, idx[:, k:k + 1])
                    nc.vector.tensor_tensor(best_s, scores[:, k:k + 1], best_s,
                                            op=ALU.max)

                # Next-step input-side pre-activations.
                gi_cur = gather_gi(best_i[:, 0:1])

    nc.compile()
    return nc


def _prep_inputs(inputs):
    """Host-side layout prep: transposes, padding, hi/lo splits, packing."""
    word_embs = np.asarray(inputs["word_embs"], dtype=np.float32)
    obj_feats = np.asarray(inputs["obj_feats"], dtype=np.float32)
    W_map = np.asarray(inputs["W_map"], dtype=np.float32)
    W_ih = np.asarray(inputs["W_ih"], dtype=np.float32)
    W_hh = np.asarray(inputs["W_hh"], dtype=np.float32)
    W_cls = np.asarray(inputs["W_cls"], dtype=np.float32)
    emb_table = np.asarray(inputs["emb_table"], dtype=np.float32)
    b_ih = np.asarray(inputs["b_ih"], dtype=np.float32)
    b_hh = np.asarray(inputs["b_hh"], dtype=np.float32)
    b_cls = np.asarray(inputs["b_cls"], dtype=np.float32)
    b_map = np.asarray(inputs["b_map"], dtype=np.float32)

    whhT = np.ascontiguousarray(W_hh.T)          # [H, 3H]

    shared = {}
    for name, w in (("whh", whhT), ("wmp", W_map)):
        for i, part in enumerate(_split3(w)):
            shared[f"{name}{i}"] = part
    wcl1 = np.zeros((H, VP), np.float16)
    wcl1[:, :V] = _split3(W_cls)[0]
    shared["wcl1"] = wcl1
    shared["wclsT"] = np.ascontiguousarray(W_cls.T)

    # Input-side GRU pre-activations, f64 on the host: [V+B, 3H] where the
    # last B rows are the SOS step (word_embs[:, 0]); split per gate.
    gi_all = np.concatenate([emb_table, word_embs[:, 0, :]], axis=0)
    gi_tab = (gi_all.astype(np.float64) @ W_ih.T.astype(np.float64)
              + b_ih.astype(np.float64)).astype(np.float32)
    for i, g in enumerate(("r", "z", "n")):
        shared[f"gi_{g}"] = np.ascontiguousarray(gi_tab[:, i * H:(i + 1) * H])
    # Step-0 ids: row r = p_local*B + b -> virtual token V+b.
    shared["idx0"] = np.ascontiguousarray(
        (V + np.tile(np.arange(B, dtype=np.uint32), PL))[:, None])

    nonzero_bias = {}
    for bname, val in (("b_rz_hh", b_hh[:2 * H]), ("b_hn", b_hh[2 * H:]),
                       ("b_cls", b_cls), ("b_map", b_map)):
        if np.any(val):
            nonzero_bias[bname] = True
            shared[bname] = np.ascontiguousarray(val[None, :], dtype=np.float32)
    if nonzero_bias.get("b_cls"):
        shared["b_clsT"] = np.ascontiguousarray(b_cls[:, None], dtype=np.float32)

    in_maps = []
    for c in range(NCORES):
        m = dict(shared)
        sl = obj_feats[:, c * PL:(c + 1) * PL]           # [B, PL, F]
        objT = np.ascontiguousarray(
            np.transpose(sl, (2, 1, 0)).reshape(F, R))   # col r = pl*B + b
        for i, part in enumerate(_split3(objT)):
            m[f"obj{i}"] = _pack_chunks(part, KF)
        in_maps.append(m)
    return in_maps, nonzero_bias


TRACE = False          # test-harness hook: set True to capture an NTFF trace
LAST_RESULTS = None


def kernel(**inputs):
    global LAST_RESULTS
    in_maps, nonzero_bias = _prep_inputs(inputs)
    key = tuple(sorted(nonzero_bias))
    if key not in _CACHE:
        _CACHE[key] = _build_program(nonzero_bias)
    nc = _CACHE[key]
    res = run_bass_kernel_spmd(nc, in_maps, core_ids=list(range(NCORES)),
                               trace=TRACE)
    LAST_RESULTS = res
    full = np.empty((B, P, NSTEP, V), np.float32)
    for c in range(NCORES):
        o = res.results[c]["out"].reshape(NSTEP, PL, B, V)
        full[:, c * PL:(c + 1) * PL] = np.transpose(o, (2, 1, 0, 3))
    return full
